# revision 1
# baseline (speedup 1.0000x reference)
"""Trainium2 Bass kernel for nn_CHAN_without_SA (conv/attention/deconv scorer).

Full-input contract: kernel(**inputs) takes the complete unsharded inputs,
shards data-parallel over batch*max_seg_num across 8 NeuronCores (10 sequences
per core; each core's sequences all belong to one batch element), runs one SPMD
Bass/Tile program, and reassembles the full output.

Device program per core (all matmuls bf16 with fp32 PSUM accumulation):
  conv1 (k=5, 2048->512) + maxpool2 : tap-accumulated shifted-window matmuls
  conv2 (k=5,  512->256) + maxpool2 : same
  additive attention x2 concepts    : kproj matmul, tanh(+q), score matmul,
                                      masked softmax, weighted sum via DVE
  deconvs (k=4,s=2,p=1) x2          : even/odd output-phase matmuls
  final score                       : folded projection  sigmoid(v . d2 + b)
where v = w_sim1^T ((w_sim2 @ concept) * w_mlp[0]) collapses the SDIM=1024
projection exactly (algebraic identity, per batch element).
"""
import numpy as np
import ml_dtypes

BF16 = ml_dtypes.bfloat16

B, M, L = 4, 20, 200
IN_C, C1, C2 = 2048, 512, 256
CDIM, DC1, DC2, SDIM = 300, 512, 256, 1024
NEG = -1e15
Lq = L // 4           # 50
NCORES = 8
SEQ = 10              # sequences per core
PAIRS = 5
K1, K2 = 16, 4        # contraction tiles for conv1 (2048/128) and conv2 (512/128)
M1, M2 = 4, 2         # output tiles for conv1 (512/128) and conv2 (256/128)
KD1, MD1 = 6, 4       # deconv1: 768/128 in, 512/128 out
KD2, MD2 = 4, 2       # deconv2: 512/128 in, 256/128 out


def _build_program():
    import concourse.bass as bass
    import concourse.mybir as mybir
    import concourse.tile as tile
    from concourse import bacc
    from contextlib import ExitStack

    dt = mybir.dt
    f32, bf16 = dt.float32, dt.bfloat16
    AF = mybir.ActivationFunctionType
    ALU = mybir.AluOpType

    nc = bacc.Bacc()
    P = nc.declare_dram_parameter
    d_xw = P("xw", [PAIRS * 8, 128, 816], bf16, isOutput=False)
    d_w1 = P("w1t", [K1, 128, 5 * 512], bf16, isOutput=False)
    d_b1 = P("b1", [M1, 128, 1], f32, isOutput=False)
    d_w2 = P("w2t", [K2, 128, 5 * 256], bf16, isOutput=False)
    d_b2 = P("b2", [M2, 128, 1], f32, isOutput=False)
    d_wca2 = P("wca2t", [2, 128, 256], bf16, isOutput=False)
    d_wca3 = P("wca3t", [2, 128, 1], bf16, isOutput=False)
    d_qv = P("qv", [4, 128, 1], f32, isOutput=False)
    d_mask = P("amask", [PAIRS, 1, 2 * Lq], f32, isOutput=False)
    d_wd1 = P("wd1t", [KD1, 128, 4 * 512], bf16, isOutput=False)
    d_bd1 = P("bd1", [MD1, 128, 1], f32, isOutput=False)
    d_wd2 = P("wd2t", [KD2, 128, 4 * 256], bf16, isOutput=False)
    d_bd2 = P("bd2", [MD2, 128, 1], f32, isOutput=False)
    d_wsum = P("wsum", [8, 128, 512], bf16, isOutput=False)
    d_v = P("vmat", [2, 128, 2], bf16, isOutput=False)
    d_bmlp = P("bmlp", [2, 1], f32, isOutput=False)
    d_out = P("out", [2, SEQ * L], f32, isOutput=True)

    with ExitStack() as ctx:
        tc = ctx.enter_context(tile.TileContext(nc))
        wp = ctx.enter_context(tc.tile_pool(name="weights", bufs=1))
        ap_ = ctx.enter_context(tc.tile_pool(name="acts", bufs=1))
        tp = ctx.enter_context(tc.tile_pool(name="trans", bufs=2))
        pp = ctx.enter_context(tc.tile_pool(name="psum", bufs=1, space="PSUM"))
        dp = ctx.enter_context(tc.tile_pool(name="drambounce", bufs=1, space="DRAM"))

        _eng_ctr = [0]

        def bulk_eng():
            # alternate the two HWDGE issue engines (SP / ACT) so bulk loads
            # use both hardware queue sets in parallel
            _eng_ctr[0] += 1
            return nc.sync if _eng_ctr[0] % 2 == 0 else nc.scalar

        def wtile(src, i, shape, dtyp, tag, small=False):
            t = wp.tile(shape, dtyp, tag=tag, name=tag)
            # small constants go via SWDGE: one queue sem per DMA, so their
            # consumers (ACT/DVE) don't blow the per-inst sync-wait budget
            eng = nc.gpsimd if small else bulk_eng()
            eng.dma_start(out=t, in_=src[i] if i is not None else src[:])
            return t

        def xtile(p, k2):
            # two conv1 k-tiles per DMA: 1632B per-partition chunks
            xk = tp.tile([128, 816], bf16, tag=f"x{k2}", name=f"x{k2}", bufs=2)
            bulk_eng().dma_start(out=xk, in_=d_xw[p * 8 + k2])
            return xk

        # ---- DMA issue order is the kernel head's critical path: per k-step
        # emit the pair-0/pair-1 x tiles around their 5-tap conv1 weight tile
        # (exact consumption order) so the conv1 passes stream behind the DMA.
        xs_pre = {0: [None] * 8, 1: [None] * 8}
        w1k = [None] * K1
        w1s = {}
        for k2 in range(8):
            xs_pre[0][k2] = xtile(0, k2)
            for k in (2 * k2, 2 * k2 + 1):
                if k < 2:
                    # per-tap loads for the first k-tiles: the very first
                    # matmul only has to wait for a 128KB transfer
                    w1s[k] = []
                    for t in range(5):
                        wt = wp.tile([128, 512], bf16, tag=f"w1s_{k}_{t}",
                                     name=f"w1s_{k}_{t}")
                        bulk_eng().dma_start(
                            out=wt, in_=d_w1[k, :, t * 512:(t + 1) * 512])
                        w1s[k].append(wt)
                else:
                    w1k[k] = wtile(d_w1, k, [128, 5 * 512], bf16, f"w1k_{k}")
            xs_pre[1][k2] = xtile(1, k2)
        w2k = [wtile(d_w2, k, [128, 5 * 256], bf16, f"w2k_{k}") for k in range(K2)]
        wca2 = [wtile(d_wca2, k, [128, 256], bf16, f"wca2_{k}") for k in range(2)]
        wca3 = [wtile(d_wca3, k, [128, 1], bf16, f"wca3_{k}", small=True) for k in range(2)]
        qv = [[wtile(d_qv, c * 2 + k, [128, 1], f32, f"qv_{c}_{k}", small=True)
               for k in range(2)] for c in range(2)]
        b1 = [wtile(d_b1, m, [128, 1], f32, f"b1_{m}", small=True) for m in range(M1)]
        b2 = [wtile(d_b2, m, [128, 1], f32, f"b2_{m}", small=True) for m in range(M2)]
        bd1 = [wtile(d_bd1, m, [128, 1], f32, f"bd1_{m}", small=True) for m in range(MD1)]
        bd2 = [wtile(d_bd2, m, [128, 1], f32, f"bd2_{m}", small=True) for m in range(MD2)]
        vm = [wtile(d_v, k, [128, 2], bf16, f"v_{k}", small=True) for k in range(2)]
        bmlp = wtile(d_bmlp, None, [2, 1], f32, "bmlp", small=True)
        mkp = [wtile(d_mask, p, [1, 2 * Lq], f32, f"mask{p}", small=True)
               for p in range(PAIRS)]

        # ---- persistent activation tiles ----
        # cat: [t2(2) | r1(2) | r2(2)] k-tiles, 10 seqs x 52 cols (1 zero pad each side)
        cat = [ap_.tile([128, SEQ * 52], bf16, tag=f"cat{j}", name=f"cat{j}") for j in range(2)]
        for t in cat:
            nc.gpsimd.memset(t, 0.0)
        d1p = [ap_.tile([128, SEQ * 102], bf16, tag=f"d1p{m}", name=f"d1p{m}") for m in range(MD1)]
        for t in d1p:
            nc.gpsimd.memset(t, 0.0)
        d2sb = [ap_.tile([128, SEQ * 200], bf16, tag=f"d2_{m}", name=f"d2_{m}") for m in range(MD2)]
        rcol = [[ap_.tile([128, SEQ], f32, tag=f"rcol{c}{k}", name=f"rcol{c}{k}") for k in range(2)]
                for c in range(2)]

        wd1k = [None] * KD1
        wd2k = [None] * KD2
        wsum = [None] * 8

        # ============ conv + attention (incl. softmax + r), per pair ========
        for p in range(PAIRS):
            if p >= 1 and p + 1 < PAIRS:
                xs_pre[p + 1] = [xtile(p + 1, k2) for k2 in range(8)]
            if p == 2:
                # deconv weights are needed only after the conv phase; issue
                # them here so they queue behind the x prefetches they'd
                # otherwise starve
                for k in range(KD1):
                    wd1k[k] = wtile(d_wd1, k, [128, 4 * 512], bf16, f"wd1k_{k}")
                for k in range(KD2):
                    wd2k[k] = wtile(d_wd2, k, [128, 4 * 256], bf16, f"wd2k_{k}")
                for j in range(8):
                    wsum[j] = wtile(d_wsum, j, [128, 512], bf16, f"wsum_{j}")
            xs = xs_pre[p]

            # conv1: k,t outer / m inner -> each weight tile is consumed for
            # all 4 output tiles as soon as it lands (head-of-kernel overlap)
            psg = [pp.tile([128, 400], f32, tag="mm400", name="mm400", bufs=4)
                   for _ in range(M1)]
            n = 0
            for k in range(K1):
                k2, k01 = divmod(k, 2)
                rv = xs[k2].rearrange("q (g s c) -> q g s c", g=2, s=2)
                for t in range(5):
                    lh = (w1s[k][t][:, :] if k < 2 else
                          w1k[k][:, t * 512:(t + 1) * 512])
                    for m in range(M1):
                        nc.tensor.matmul(
                            psg[m], lhsT=lh[:, m * 128:(m + 1) * 128],
                            rhs=rv[:, k01, :, t:t + 200],
                            start=(n == 0), stop=(n == 5 * K1 - 1))
                    n += 1
            t1 = []
            for m in range(M1):
                ps = psg[m]
                t1m = tp.tile([128, 2 * 104], bf16, tag=f"t1_{m}", name=f"t1_{m}", bufs=2)
                tmp = tp.tile([128, 200], f32, tag="ptmp1", name="ptmp1", bufs=3)
                pr = ps.rearrange("q (s l two) -> q s l two", s=2, two=2)
                tv = tmp.rearrange("q (s l) -> q s l", s=2)
                # pool+bias: max(even+b, odd+b); only one PSUM input per inst
                nc.scalar.activation(out=tv, in_=pr[:, :, :, 0],
                                     func=AF.Identity, bias=b1[m], scale=1.0)
                nc.gpsimd.memset(t1m, 0.0)
                nc.vector.scalar_tensor_tensor(
                    out=t1m.rearrange("q (s c) -> q s c", s=2)[:, :, 2:102],
                    in0=pr[:, :, :, 1], scalar=b1[m], in1=tv,
                    op0=ALU.add, op1=ALU.max)
                t1.append(t1m)

            # conv2 + pool -> t2 part of cat
            for m in range(M2):
                ps = pp.tile([128, 200], f32, tag="mm200", name="mm200", bufs=2)
                n = 0
                for k in range(K2):
                    rv = t1[k].rearrange("q (s c) -> q s c", s=2)
                    for t in range(5):
                        nc.tensor.matmul(
                            ps,
                            lhsT=w2k[k][:, t * 256 + m * 128:t * 256 + (m + 1) * 128],
                            rhs=rv[:, :, t:t + 100],
                            start=(n == 0), stop=(n == 5 * K2 - 1))
                        n += 1
                tmp = tp.tile([128, 100], f32, tag="ptmp2", name="ptmp2", bufs=3)
                pr = ps.rearrange("q (s l two) -> q s l two", s=2, two=2)
                tv = tmp.rearrange("q (s l) -> q s l", s=2)
                nc.scalar.activation(out=tv, in_=pr[:, :, :, 0],
                                     func=AF.Identity, bias=b2[m], scale=1.0)
                nc.vector.scalar_tensor_tensor(
                    out=cat[m].rearrange("q (s c) -> q s c", s=SEQ)[
                        :, 2 * p:2 * p + 2, 1:1 + Lq],
                    in0=pr[:, :, :, 1], scalar=b2[m], in1=tv,
                    op0=ALU.add, op1=ALU.max)

            # attention for this pair (runs on ACT/DVE/DMA under the next
            # pair's conv1 on PE)
            kp = []
            for m in range(M2):
                kpm = pp.tile([128, 100], f32, tag="mm200", name="mm200", bufs=2)
                for k in range(2):
                    nc.tensor.matmul(
                        kpm, lhsT=wca2[k][:, m * 128:(m + 1) * 128],
                        rhs=cat[k].rearrange("q (s c) -> q s c", s=SEQ)[
                            :, 2 * p:2 * p + 2, 1:1 + Lq],
                        start=(k == 0), stop=(k == 1))
                kp.append(kpm)
            for c in range(2):
                th = []
                for m in range(M2):
                    thm = tp.tile([128, 100], bf16, tag=f"th{c}{m}", name=f"th{c}{m}", bufs=2)
                    nc.scalar.activation(out=thm, in_=kp[m], func=AF.Tanh,
                                         bias=qv[c][m], scale=1.0)
                    th.append(thm)
                sp = pp.tile([1, 100], f32, tag="tiny", name="tiny", bufs=1)
                for m in range(M2):
                    nc.tensor.matmul(sp, lhsT=wca3[m], rhs=th[m],
                                     start=(m == 0), stop=(m == 1))
                # masked softmax in flat [1, 100] layout (2 blocks of 50);
                # per-block broadcasts use 0-stride AP reads on DVE
                def bc2(t):
                    return bass.AP(tensor=t.tensor, offset=t.offset,
                                   ap=[t.ap[0], [1, 2], [0, Lq]])
                sfl = tp.tile([1, 100], f32, tag="sfl", name="sfl", bufs=4)
                nc.vector.tensor_copy(out=sfl, in_=sp[0:1, 0:100])
                sm = tp.tile([1, 100], f32, tag="sm", name="sm", bufs=4)
                nc.vector.tensor_add(sm, sfl, mkp[p])
                smv = sm.rearrange("q (s l) -> q s l", s=2)
                mx = tp.tile([1, 2], f32, tag="mx", name="mx", bufs=4)
                nc.vector.tensor_reduce(out=mx, in_=smv,
                                        axis=mybir.AxisListType.X, op=ALU.max)
                sub = tp.tile([1, 100], f32, tag="sub", name="sub", bufs=4)
                nc.vector.tensor_sub(sub.rearrange("q (s l) -> q s l", s=2),
                                     smv, bc2(mx))
                ex = tp.tile([1, 100], f32, tag="ex", name="ex", bufs=4)
                nc.scalar.activation(out=ex, in_=sub, func=AF.Exp,
                                     bias=0.0, scale=1.0)
                exv = ex.rearrange("q (s l) -> q s l", s=2)
                se = tp.tile([1, 2], f32, tag="se", name="se", bufs=4)
                nc.vector.tensor_reduce(out=se, in_=exv,
                                        axis=mybir.AxisListType.X, op=ALU.add)
                rc = tp.tile([1, 2], f32, tag="rc", name="rc", bufs=4)
                nc.vector.reciprocal(rc, se)
                av = tp.tile([1, 100], f32, tag="av", name="av", bufs=4)
                nc.vector.tensor_mul(av.rearrange("q (s l) -> q s l", s=2),
                                     exv, bc2(rc))
                # broadcast attention weights to all partitions (no DRAM hop)
                abc2 = tp.tile([128, 100], f32, tag="abc", name="abc", bufs=4)
                nc.gpsimd.partition_broadcast(abc2, av[0:1, :], channels=128)
                for s01 in range(2):
                    s = 2 * p + s01
                    abc = abc2[:, Lq * s01:Lq * s01 + Lq]
                    for k in range(2):
                        scr = tp.tile([128, Lq], f32, tag="rscr", name="rscr", bufs=2)
                        nc.vector.tensor_mul(
                            scr, cat[k][:, 52 * s + 1:52 * s + 1 + Lq], abc)
                        nc.vector.tensor_reduce(
                            out=rcol[c][k][:, s:s + 1], in_=scr,
                            axis=mybir.AxisListType.X, op=ALU.add)

        # ================= deconv1: cat[768,50] -> d1[512,100] ==============
        # even out o=2j: tap1@U[j] + tap3@U[j-1]; odd o=2j+1: tap2@U[j] + tap0@U[j+1]
        # r-channels (256..767) are constant along j, so their contribution is
        # rank-1 per sequence: broadcast((Wt_a+Wt_b)^T r) with a single-column
        # boundary correction (j=0 even: -W3^T r; j=49 odd: -W0^T r).
        rcolb = []
        for c in range(2):
            for k in range(2):
                rb = ap_.tile([128, SEQ], bf16, tag=f"rcolb{c}{k}",
                              name=f"rcolb{c}{k}")
                nc.vector.tensor_copy(out=rb, in_=rcol[c][k])
                rcolb.append(rb)          # rk = 2*c + k matches cat channel order
        rt = pp.tile([128, 320], f32, tag="rt", name="rt", bufs=1)
        for phase in range(2):
            corr_t = 3 if phase == 0 else 0
            for m in range(MD1):
                g = phase * 4 + m
                for rk in range(4):
                    nc.tensor.matmul(
                        rt[:, g * 20:g * 20 + 10],
                        lhsT=wsum[phase * 4 + rk][:, m * 128:(m + 1) * 128],
                        rhs=rcolb[rk], start=(rk == 0), stop=(rk == 3))
                for rk in range(4):
                    nc.tensor.matmul(
                        rt[:, g * 20 + 10:g * 20 + 20],
                        lhsT=wd1k[2 + rk][:, corr_t * 512 + m * 128:
                                          corr_t * 512 + (m + 1) * 128],
                        rhs=rcolb[rk], start=(rk == 0), stop=(rk == 3))
        for m in range(MD1):
            for phase, taps in enumerate([[(1, 1), (3, 0)], [(2, 1), (0, 2)]]):
                ps = pp.tile([128, SEQ * Lq], f32, tag="mm400", name="mm400", bufs=4)
                n = 0
                for k in range(2):
                    for t, off in taps:
                        nc.tensor.matmul(
                            ps,
                            lhsT=wd1k[k][:, t * 512 + m * 128:t * 512 + (m + 1) * 128],
                            rhs=cat[k].rearrange("q (s c) -> q s c", s=SEQ)[
                                :, :, off:off + Lq],
                            start=(n == 0), stop=(n == 3))
                        n += 1
                g = phase * 4 + m
                ert = tp.tile([128, 20], f32, tag="ert", name="ert", bufs=2)
                nc.vector.tensor_copy(out=ert, in_=rt[:, g * 20:g * 20 + 20])
                er2 = tp.tile([128, SEQ], f32, tag="er2", name="er2", bufs=2)
                nc.vector.tensor_sub(er2, ert[:, 0:10], ert[:, 10:20])
                dst = d1p[m].rearrange("q (s c) -> q s c", s=SEQ)[:, :, 1:101] \
                    .rearrange("q s (l two) -> q s l two", two=2)[:, :, :, phase]
                erb = bass.AP(tensor=ert.tensor, offset=ert.offset,
                              ap=[ert.ap[0], [1, SEQ], [0, Lq]])
                nc.vector.scalar_tensor_tensor(
                    out=dst, in0=ps.rearrange("q (s l) -> q s l", s=SEQ),
                    scalar=bd1[m], in1=erb, op0=ALU.add, op1=ALU.add)
                bcol = 0 if phase == 0 else Lq - 1
                nc.vector.scalar_tensor_tensor(
                    out=dst[:, :, bcol:bcol + 1],
                    in0=ps.rearrange("q (s l) -> q s l", s=SEQ)[:, :, bcol:bcol + 1],
                    scalar=bd1[m],
                    in1=er2.rearrange("q (s o) -> q s o", o=1),
                    op0=ALU.add, op1=ALU.add)

        # ================= deconv2: d1[512,100] -> d2[256,200] ==============
        for m in range(MD2):
            for g in range(2):          # groups of 5 seqs (PSUM bank limit)
                for phase, taps in enumerate([[(1, 1), (3, 0)], [(2, 1), (0, 2)]]):
                    ps = pp.tile([128, 500], f32, tag="mm400", name="mm400", bufs=4)
                    n = 0
                    for k in range(KD2):
                        for t, off in taps:
                            nc.tensor.matmul(
                                ps,
                                lhsT=wd2k[k][:, t * 256 + m * 128:t * 256 + (m + 1) * 128],
                                rhs=d1p[k].rearrange("q (s c) -> q s c", s=SEQ)[
                                    :, 5 * g:5 * g + 5, off:off + 100],
                                start=(n == 0), stop=(n == 2 * KD2 - 1))
                            n += 1
                    dst = d2sb[m].rearrange("q (s c) -> q s c", s=SEQ)[
                        :, 5 * g:5 * g + 5, :] \
                        .rearrange("q s (l two) -> q s l two", two=2)[:, :, :, phase]
                    nc.scalar.activation(
                        out=dst, in_=ps.rearrange("q (s l) -> q s l", s=5),
                        func=AF.Identity, bias=bd2[m], scale=1.0)

        # ================= folded final projection + sigmoid ================
        for p in range(PAIRS):
            fp = pp.tile([2, 400], f32, tag="tiny", name="tiny", bufs=1)
            for k in range(2):
                nc.tensor.matmul(fp, lhsT=vm[k],
                                 rhs=d2sb[k][:, 400 * p:400 * (p + 1)],
                                 start=(k == 0), stop=(k == 1))
            fo = tp.tile([2, 400], f32, tag="fout", name="fout", bufs=5)
            nc.scalar.activation(out=fo, in_=fp, func=AF.Sigmoid,
                                 bias=bmlp, scale=1.0)
            nc.sync.dma_start(out=d_out[:, 400 * p:400 * (p + 1)], in_=fo)

    nc.compile()   # bacc legalization: splits sync waits to <=1 per inst
    return nc


def _prep_inputs(batch, seg_len, concept1, concept2,
                 w_conv1, b_conv1, w_conv2, b_conv2,
                 w_ca1, w_ca2, w_ca3,
                 w_dc1, b_dc1, w_dc2, b_dc2,
                 w_sim1, w_sim2, w_mlp, b_mlp):
    f32 = np.float32

    # x: [B,M,L,IN_C] -> per core [PAIRS*K1, 128, 408] padded pairs (bf16)
    bm = np.ascontiguousarray(batch, f32).reshape(B * M, L, IN_C)
    bt = bm.transpose(0, 2, 1).astype(BF16)            # [80, 2048, 200]
    X = np.zeros((B * M, K1, 128, 204), BF16)
    X[:, :, :, 2:202] = bt.reshape(B * M, K1, 128, L)
    # pack [pair, k2(8), 128, (k01, seq01, 204)] = [.., 128, 816]
    xw = X.reshape(NCORES, PAIRS, 2, 8, 2, 128, 204) \
          .transpose(0, 1, 3, 5, 4, 2, 6) \
          .reshape(NCORES, PAIRS * 8, 128, 816)
    xw = np.ascontiguousarray(xw)

    # weight layouts: one DMA per contraction k-tile holding all taps
    # [k, ci, (t, m, co)] -> big per-partition contiguous chunks
    w1t = np.ascontiguousarray(
        np.asarray(w_conv1, f32).reshape(M1, 128, K1, 128, 5)
        .transpose(2, 3, 4, 0, 1).reshape(K1, 128, 5 * 512)).astype(BF16)
    w2t = np.ascontiguousarray(
        np.asarray(w_conv2, f32).reshape(M2, 128, K2, 128, 5)
        .transpose(2, 3, 4, 0, 1).reshape(K2, 128, 5 * 256)).astype(BF16)
    wd1t = np.ascontiguousarray(
        np.asarray(w_dc1, f32).reshape(KD1, 128, MD1, 128, 4)
        .transpose(0, 1, 4, 2, 3).reshape(KD1, 128, 4 * 512)).astype(BF16)
    wd2t = np.ascontiguousarray(
        np.asarray(w_dc2, f32).reshape(KD2, 128, MD2, 128, 4)
        .transpose(0, 1, 4, 2, 3).reshape(KD2, 128, 4 * 256)).astype(BF16)
    wca2t = np.ascontiguousarray(np.asarray(w_ca2, f32).T.reshape(2, 128, 256)) \
        .astype(BF16)
    # summed-tap deconv1 weights for the broadcast r-channels: [ph*4+rk, ci, (m co)]
    wr = np.asarray(w_dc1, f32)[256:768].reshape(4, 128, MD1, 128, 4)
    wsum = np.ascontiguousarray(
        np.stack([wr[..., 1] + wr[..., 3], wr[..., 2] + wr[..., 0]], 0)
        .reshape(8, 128, 512)).astype(BF16)
    wca3t = np.asarray(w_ca3, f32)[0].reshape(2, 128, 1).astype(BF16)
    b1 = np.asarray(b_conv1, f32).reshape(M1, 128, 1)
    b2 = np.asarray(b_conv2, f32).reshape(M2, 128, 1)
    bd1v = np.asarray(b_dc1, f32).reshape(MD1, 128, 1)
    bd2v = np.asarray(b_dc2, f32).reshape(MD2, 128, 1)
    bmlp = np.full((2, 1), np.asarray(b_mlp, f32).reshape(-1)[0], f32)

    # per-core mask / q / v
    nvalid = ((np.asarray(seg_len) + 3) // 4).reshape(B * M)
    amask = np.where(np.arange(Lq)[None, :] < nvalid[:, None], 0.0, NEG) \
        .astype(f32).reshape(NCORES, PAIRS, 1, 2 * Lq)
    concepts = [np.asarray(concept1, f32), np.asarray(concept2, f32)]
    w_ca1 = np.asarray(w_ca1, f32)
    w_sim1 = np.asarray(w_sim1, f32)
    w_sim2 = np.asarray(w_sim2, f32)
    wm = np.asarray(w_mlp, f32)[0]
    qv_all = np.zeros((NCORES, 4, 128, 1), f32)
    v_all = np.zeros((NCORES, 2, 128, 2), f32)
    for core in range(NCORES):
        bidx = (core * SEQ) // M
        for c in range(2):
            q = w_ca1 @ concepts[c][bidx]                       # [256]
            qv_all[core, 2 * c:2 * c + 2] = q.reshape(2, 128, 1)
            v = w_sim1.T @ ((w_sim2 @ concepts[c][bidx]) * wm)  # [256]
            v_all[core, :, :, c] = v.reshape(2, 128)
    vmat = v_all.astype(BF16)

    shared = dict(w1t=w1t, b1=b1, wsum=wsum, w2t=w2t, b2=b2, wca2t=wca2t, wca3t=wca3t,
                  wd1t=wd1t, bd1=bd1v, wd2t=wd2t, bd2=bd2v, bmlp=bmlp)
    return [dict(shared, xw=xw[c], amask=amask[c], qv=qv_all[c], vmat=vmat[c])
            for c in range(NCORES)]


_CACHE = {}


def kernel(**inputs):
    from concourse.bass_utils import run_bass_kernel_spmd

    in_maps = _prep_inputs(**inputs)
    if "nc" not in _CACHE:
        _CACHE["nc"] = _build_program()
    res = run_bass_kernel_spmd(_CACHE["nc"], in_maps, list(range(NCORES)))
    out = np.stack([np.asarray(r["out"], np.float32) for r in res.results])
    sc = out.transpose(1, 0, 2).reshape(2, B, M, L)
    return sc[0], sc[1]



# revision 2
# speedup vs baseline: 1.7181x; 1.7181x over previous
"""Trainium2 Bass kernel for nn_CHAN_without_SA (conv/attention/deconv scorer).

Full-input contract: kernel(**inputs) takes the complete unsharded inputs,
shards data-parallel over batch*max_seg_num across 8 NeuronCores (10 sequences
per core; each core's sequences all belong to one batch element), runs one SPMD
Bass/Tile program, and reassembles the full output.

FP8 (e4m3) DoubleRow edition: conv1/conv2/deconv1/deconv2 matmuls run with
perf_mode=DoubleRow (2 fp8 weights per PE cell, contraction 256/instruction,
~1.4-2x bf16 throughput). Weights are pre-scaled by a power of two so their
~0.02-sigma values land in the fp8 normal range; activations are rescaled at
each stage output via the activation-unit scale operand (exact).

Layout tricks:
  - conv rhs streams a flat multi-sequence window (FD 408/412/508 >= 256) so
    the doubled LDWEIGHTS stays hidden behind the matmul; junk columns between
    sequence regions are simply never read out of PSUM.
  - deconv1's broadcast r-channels stay rank-1 (wsum/wcorr folded matmuls).
  - the SDIM=1024 final projection is folded to v = w_sim1^T((w_sim2 c) * w_mlp)
    exactly, per batch element.
"""
import numpy as np
import ml_dtypes

BF16 = ml_dtypes.bfloat16
F8 = ml_dtypes.float8_e4m3      # IEEE-style e4m3: matches TRN FP8_EXP4 on [0,240]

B, M, L = 4, 20, 200
IN_C, C1, C2 = 2048, 512, 256
CDIM, DC1, DC2, SDIM = 300, 512, 256, 1024
NEG = -1e15
Lq = L // 4           # 50
NCORES = 8
SEQ = 10              # sequences per core
PAIRS = 5
M1, M2 = 4, 2         # output 128-tiles for conv1 / conv2
MD1, MD2 = 4, 2       # output 128-tiles for deconv1 / deconv2
BLOCKS = [(0, 1), (2, 3), (4,)]   # pair blocks (conv2 batches 2 pairs)
# power-of-two weight scales: w*S lands sigma~0.3 in the fp8 normal range
S1, S2, SA, SD1, SD2 = 8.0, 16.0, 16.0, 16.0, 16.0


def _build_program():
    import concourse.bass as bass
    import concourse.mybir as mybir
    import concourse.tile as tile
    from concourse import bacc
    from contextlib import ExitStack

    dt = mybir.dt
    f32, bf16, f8 = dt.float32, dt.bfloat16, dt.float8e4
    AF = mybir.ActivationFunctionType
    ALU = mybir.AluOpType
    DR = mybir.MatmulPerfMode.DoubleRow
    X_AX = mybir.AxisListType.X

    nc = bacc.Bacc()
    P = nc.declare_dram_parameter
    d_xw = P("xw", [PAIRS, 128, 6656], f8, isOutput=False)
    d_w1 = P("w1", [8, 128, 5120], f8, isOutput=False)
    d_w2 = P("w2", [2, 128, 2560], f8, isOutput=False)
    d_wd1 = P("wd1", [128, 4096], f8, isOutput=False)
    d_wd2 = P("wd2", [2, 128, 2048], f8, isOutput=False)
    d_wsum = P("wsum", [128, 4096], f8, isOutput=False)
    d_wcorr = P("wcorr", [128, 4096], f8, isOutput=False)
    d_wca2 = P("wca2", [2, 128, 256], f8, isOutput=False)
    d_wca3 = P("wca3", [2, 128, 1], bf16, isOutput=False)
    d_qv = P("qv", [4, 128, 1], f32, isOutput=False)
    d_b1s = P("b1s", [M1, 128, 1], f32, isOutput=False)
    d_b2 = P("b2", [M2, 128, 1], f32, isOutput=False)
    d_bd1 = P("bd1", [MD1, 128, 1], f32, isOutput=False)
    d_bd2 = P("bd2", [MD2, 128, 1], f32, isOutput=False)
    d_mask = P("amask", [PAIRS, 1, 2 * Lq], f32, isOutput=False)
    d_v = P("vmat", [2, 128, 2], bf16, isOutput=False)
    d_bmlp = P("bmlp", [2, 1], f32, isOutput=False)
    d_out = P("out", [2, SEQ * L], f32, isOutput=True)

    with ExitStack() as ctx:
        tc = ctx.enter_context(tile.TileContext(nc))
        wp = ctx.enter_context(tc.tile_pool(name="weights", bufs=1))
        ap_ = ctx.enter_context(tc.tile_pool(name="acts", bufs=1))
        tp = ctx.enter_context(tc.tile_pool(name="trans", bufs=2))
        pp = ctx.enter_context(tc.tile_pool(name="psum", bufs=1, space="PSUM"))

        def apn(t, off, *dims):
            # free-dim view at element offset `off`: dims = (stride, n) pairs
            base = t[:, off:off + 1]
            return bass.AP(tensor=base.tensor, offset=base.offset,
                           ap=[base.ap[0]] + [list(d) for d in dims])

        # ---- bulk DMA on the two HWDGE issue engines, all issued up front
        # (tiles are persistent single-buffer: no reuse waits can ever park
        # in front of compute instructions on these queues)
        def wtile(eng, src, i, shape, dtyp, tag):
            t = wp.tile(shape, dtyp, tag=tag, name=tag)
            eng.dma_start(out=t, in_=src[i] if i is not None else src[:])
            return t

        xt = [None] * PAIRS
        w1sb = [None] * 8
        xt[0] = wtile(nc.sync, d_xw, 0, [128, 6656], f8, "x0")
        w1sb[0] = wtile(nc.scalar, d_w1, 0, [128, 5120], f8, "w1_0")
        xt[1] = wtile(nc.scalar, d_xw, 1, [128, 6656], f8, "x1")
        w1sb[1] = wtile(nc.sync, d_w1, 1, [128, 5120], f8, "w1_1")
        xt[2] = wtile(nc.sync, d_xw, 2, [128, 6656], f8, "x2")
        w1sb[2] = wtile(nc.scalar, d_w1, 2, [128, 5120], f8, "w1_2")
        xt[3] = wtile(nc.scalar, d_xw, 3, [128, 6656], f8, "x3")
        w1sb[3] = wtile(nc.sync, d_w1, 3, [128, 5120], f8, "w1_3")
        xt[4] = wtile(nc.sync, d_xw, 4, [128, 6656], f8, "x4")
        w1sb[4] = wtile(nc.scalar, d_w1, 4, [128, 5120], f8, "w1_4")
        w1sb[5] = wtile(nc.sync, d_w1, 5, [128, 5120], f8, "w1_5")
        w1sb[6] = wtile(nc.scalar, d_w1, 6, [128, 5120], f8, "w1_6")
        w1sb[7] = wtile(nc.sync, d_w1, 7, [128, 5120], f8, "w1_7")
        wca2sb = [wtile(nc.scalar, d_wca2, k, [128, 256], f8, f"wca2_{k}")
                  for k in range(2)]
        w2sb = [wtile(nc.sync if k == 0 else nc.scalar, d_w2, k,
                      [128, 2560], f8, f"w2_{k}") for k in range(2)]
        wd1sb = wtile(nc.sync, d_wd1, None, [128, 4096], f8, "wd1")
        wd2sb = [wtile(nc.scalar, d_wd2, k, [128, 2048], f8, f"wd2_{k}")
                 for k in range(2)]
        wsumsb = wtile(nc.sync, d_wsum, None, [128, 4096], f8, "wsum")
        wcorrsb = wtile(nc.scalar, d_wcorr, None, [128, 4096], f8, "wcorr")

        # ---- persistent activations; pads are zeroed once, never rewritten
        t1p = [ap_.tile([128, 832], f8, tag=f"t1p{k}", name=f"t1p{k}")
               for k in range(2)]
        cat8 = ap_.tile([128, 1056], f8, tag="cat8", name="cat8")
        d1p8 = [ap_.tile([128, 2048], f8, tag=f"d1p{k}", name=f"d1p{k}")
                for k in range(2)]
        d2sb = [ap_.tile([128, SEQ * 200], bf16, tag=f"d2_{m}", name=f"d2_{m}")
                for m in range(MD2)]
        rcol = [[ap_.tile([128, SEQ], f32, tag=f"rcol{c}{k}", name=f"rcol{c}{k}")
                 for k in range(2)] for c in range(2)]
        for t in t1p + [cat8] + d1p8:
            nc.gpsimd.memset(t, 0.0)

        # small constants via SWDGE (one queue sem per DMA)
        b1s = [wtile(nc.gpsimd, d_b1s, m, [128, 1], f32, f"b1s_{m}") for m in range(M1)]
        b2 = [wtile(nc.gpsimd, d_b2, m, [128, 1], f32, f"b2_{m}") for m in range(M2)]
        qv = [wtile(nc.gpsimd, d_qv, i, [128, 1], f32, f"qv_{i}") for i in range(4)]
        wca3 = [wtile(nc.gpsimd, d_wca3, k, [128, 1], bf16, f"wca3_{k}") for k in range(2)]
        mkp = [wtile(nc.gpsimd, d_mask, p, [1, 2 * Lq], f32, f"mask{p}") for p in range(PAIRS)]
        bd1 = [wtile(nc.gpsimd, d_bd1, m, [128, 1], f32, f"bd1_{m}") for m in range(MD1)]
        bd2 = [wtile(nc.gpsimd, d_bd2, m, [128, 1], f32, f"bd2_{m}") for m in range(MD2)]
        vm = [wtile(nc.gpsimd, d_v, k, [128, 2], bf16, f"v_{k}") for k in range(2)]
        bmlp = wtile(nc.gpsimd, d_bmlp, None, [2, 1], f32, "bmlp")

        # ============ conv1: x[2048,200] -> maxpool -> t1[512,100] ==========
        # DoubleRow over channel-ktile pairs; rhs streams a flat 2-seq window
        # (FD=408, junk cols [200,208) per seq region). t1 is kept S1-scaled
        # in fp8 (pool max commutes with the affine S1*x + S1*b map).
        def emit_conv1(blk, weave=()):
            prs = BLOCKS[blk]
            for m in range(M1):
                pst = [pp.tile([128, 508], f32, tag="big", name="big", bufs=4)
                       for _ in prs]
                for kp in range(8):
                    for t in range(5):
                        lhs = apn(w1sb[kp], t * 512 + m * 128, (2560, 2), (1, 128))
                        for pi in range(len(prs)):
                            nc.tensor.matmul(
                                pst[pi][:, 0:408], lhsT=lhs,
                                rhs=apn(xt[prs[pi]], kp * 832 + t, (416, 2), (1, 408)),
                                start=(kp == 0 and t == 0), stop=(kp == 7 and t == 4),
                                perf_mode=DR)
                for pi in range(len(prs)):
                    ps = pst[pi]
                    pre = apn(ps, 0, (208, 2), (2, 100))
                    pro = apn(ps, 1, (208, 2), (2, 100))
                    tv = tp.tile([128, 200], f32, tag="ptmp1", name="ptmp1", bufs=3)
                    tvv = apn(tv, 0, (100, 2), (1, 100))
                    nc.scalar.activation(out=tvv, in_=pre, func=AF.Identity,
                                         bias=b1s[m], scale=1.0)
                    dst = apn(t1p[m // 2], (m % 2) * 416 + (2 * pi) * 104 + 2,
                              (104, 2), (1, 100))
                    nc.vector.scalar_tensor_tensor(
                        out=dst, in0=pro, scalar=b1s[m], in1=tvv,
                        op0=ALU.add, op1=ALU.max)
                if m < len(weave):
                    weave[m]()

        # ============ conv2 + maxpool -> t2 (cat8, true scale) ==============
        def emit_conv2(blk):
            prs = BLOCKS[blk]
            sblk = 2 * len(prs)
            W2 = 104 * (sblk - 1) + 100
            for m in range(M2):
                ps = pp.tile([128, 508], f32, tag="big", name="big", bufs=4)
                n = 0
                for kp in range(2):
                    for t in range(5):
                        nc.tensor.matmul(
                            ps[:, 0:W2],
                            lhsT=apn(w2sb[kp], t * 256 + m * 128, (1280, 2), (1, 128)),
                            rhs=apn(t1p[kp], t, (416, 2), (1, W2)),
                            start=(n == 0), stop=(n == 9), perf_mode=DR)
                        n += 1
                pre = apn(ps, 0, (104, sblk), (2, 50))
                pro = apn(ps, 1, (104, sblk), (2, 50))
                te = tp.tile([128, 200], f32, tag="c2e", name="c2e", bufs=2)
                to = tp.tile([128, 200], f32, tag="c2o", name="c2o", bufs=2)
                tev = apn(te, 0, (50, sblk), (1, 50))
                tov = apn(to, 0, (50, sblk), (1, 50))
                inv = 1.0 / (S1 * S2)
                nc.scalar.activation(out=tev, in_=pre, func=AF.Identity,
                                     bias=b2[m], scale=inv)
                nc.scalar.activation(out=tov, in_=pro, func=AF.Identity,
                                     bias=b2[m], scale=inv)
                dst = apn(cat8, m * 528 + (4 * blk) * 52 + 1, (52, sblk), (1, 50))
                nc.vector.tensor_max(dst, tev, tov)

        # ============ additive attention for one pair =======================
        def attention(p):
            kp_ps = []
            for m in range(M2):
                kpm = pp.tile([128, 100], f32, tag="mm200", name="mm200", bufs=2)
                for k in range(2):
                    nc.tensor.matmul(
                        kpm, lhsT=wca2sb[k][:, m * 128:(m + 1) * 128],
                        rhs=apn(cat8, k * 528 + 104 * p + 1, (52, 2), (1, 50)),
                        start=(k == 0), stop=(k == 1))
                kp_ps.append(kpm)
            for c in range(2):
                th = []
                for m in range(M2):
                    thm = tp.tile([128, 100], bf16, tag=f"th{c}{m}", name=f"th{c}{m}", bufs=2)
                    nc.scalar.activation(out=thm, in_=kp_ps[m], func=AF.Tanh,
                                         bias=qv[2 * c + m], scale=1.0 / SA)
                    th.append(thm)
                sp = pp.tile([1, 100], f32, tag="tiny", name="tiny", bufs=1)
                for m in range(M2):
                    nc.tensor.matmul(sp, lhsT=wca3[m], rhs=th[m],
                                     start=(m == 0), stop=(m == 1))
                # masked softmax in flat [1, 100] layout (2 blocks of 50)
                def bc2(t):
                    return bass.AP(tensor=t.tensor, offset=t.offset,
                                   ap=[t.ap[0], [1, 2], [0, Lq]])
                sfl = tp.tile([1, 100], f32, tag="sfl", name="sfl", bufs=4)
                nc.vector.tensor_copy(out=sfl, in_=sp[0:1, 0:100])
                sm = tp.tile([1, 100], f32, tag="sm", name="sm", bufs=4)
                nc.vector.tensor_add(sm, sfl, mkp[p])
                smv = sm.rearrange("q (s l) -> q s l", s=2)
                mx = tp.tile([1, 2], f32, tag="mx", name="mx", bufs=4)
                nc.vector.tensor_reduce(out=mx, in_=smv, axis=X_AX, op=ALU.max)
                sub = tp.tile([1, 100], f32, tag="sub", name="sub", bufs=4)
                nc.vector.tensor_sub(sub.rearrange("q (s l) -> q s l", s=2),
                                     smv, bc2(mx))
                ex = tp.tile([1, 100], f32, tag="ex", name="ex", bufs=4)
                nc.scalar.activation(out=ex, in_=sub, func=AF.Exp,
                                     bias=0.0, scale=1.0)
                exv = ex.rearrange("q (s l) -> q s l", s=2)
                se = tp.tile([1, 2], f32, tag="se", name="se", bufs=4)
                nc.vector.tensor_reduce(out=se, in_=exv, axis=X_AX, op=ALU.add)
                rc = tp.tile([1, 2], f32, tag="rc", name="rc", bufs=4)
                nc.vector.reciprocal(rc, se)
                av = tp.tile([1, 100], f32, tag="av", name="av", bufs=4)
                nc.vector.tensor_mul(av.rearrange("q (s l) -> q s l", s=2),
                                     exv, bc2(rc))
                abc2 = tp.tile([128, 100], f32, tag="abc", name="abc", bufs=4)
                nc.gpsimd.partition_broadcast(abc2, av[0:1, :], channels=128)
                for s01 in range(2):
                    s = 2 * p + s01
                    abc = abc2[:, Lq * s01:Lq * s01 + Lq]
                    for k in range(2):
                        scr = tp.tile([128, Lq], f32, tag="rscr", name="rscr", bufs=2)
                        nc.vector.tensor_mul(
                            scr, apn(cat8, k * 528 + 52 * s + 1, (1, Lq)), abc)
                        nc.vector.tensor_reduce(
                            out=rcol[c][k][:, s:s + 1], in_=scr,
                            axis=X_AX, op=ALU.add)

        # ---- pipeline: previous block's attention weaves into the next
        # block's conv1 m-passes so score matmuls never stall the PE
        emit_conv1(0)
        emit_conv2(0)
        emit_conv1(1, weave=(lambda: attention(0), lambda: attention(1)))
        emit_conv2(1)
        emit_conv1(2, weave=(lambda: attention(2), lambda: attention(3)))
        emit_conv2(2)
        attention(4)

        # ============ deconv1: cat[512+r,50] -> d1[512,100] =================
        # r-channels are rank-1 per sequence: rt = folded (wsum/wcorr) matmuls
        rcolb = []
        for c in range(2):
            for k in range(2):
                rb = ap_.tile([128, SEQ], f8, tag=f"rcolb{c}{k}", name=f"rcolb{c}{k}")
                nc.vector.tensor_copy(out=rb, in_=rcol[c][k])
                rcolb.append(rb)          # rk = 2*c + k matches cat channel order
        rt = pp.tile([128, 320], f32, tag="rt", name="rt", bufs=1)
        for ph in range(2):
            for m in range(MD1):
                g = ph * 4 + m
                for rk in range(4):
                    o = (ph * 4 + rk) * 512 + m * 128
                    nc.tensor.matmul(rt[:, g * 20:g * 20 + 10],
                                     lhsT=wsumsb[:, o:o + 128], rhs=rcolb[rk],
                                     start=(rk == 0), stop=(rk == 3))
                for rk in range(4):
                    o = (ph * 4 + rk) * 512 + m * 128
                    nc.tensor.matmul(rt[:, g * 20 + 10:g * 20 + 20],
                                     lhsT=wcorrsb[:, o:o + 128], rhs=rcolb[rk],
                                     start=(rk == 0), stop=(rk == 3))

        TAPS = [[(1, 1), (3, 0)], [(2, 1), (0, 2)]]   # (tap, rhs offset) per phase
        for m in range(MD1):
            for ph in range(2):
                g20 = (ph * 4 + m) * 20
                ertm = tp.tile([128, 10], f32, tag="ertm", name="ertm", bufs=2)
                nc.scalar.activation(out=ertm, in_=rt[:, g20:g20 + 10],
                                     func=AF.Identity, bias=bd1[m], scale=1.0 / SD1)
                ertc = tp.tile([128, 10], f32, tag="ertc", name="ertc", bufs=2)
                nc.scalar.activation(out=ertc, in_=rt[:, g20 + 10:g20 + 20],
                                     func=AF.Identity, bias=0.0, scale=1.0 / SD1)
                er2 = tp.tile([128, 10], f32, tag="er2", name="er2", bufs=2)
                nc.vector.tensor_sub(er2, ertm, ertc)
                psd = [pp.tile([128, 508], f32, tag="big", name="big", bufs=4)
                       for _ in range(2)]
                for ti, (t, off) in enumerate(TAPS[ph]):
                    lhs = apn(wd1sb, t * 512 + m * 128, (2048, 2), (1, 128))
                    for gi in range(2):
                        nc.tensor.matmul(
                            psd[gi][:, 0:258], lhsT=lhs,
                            rhs=apn(cat8, 260 * gi + off, (528, 2), (1, 258)),
                            start=(ti == 0), stop=(ti == 1), perf_mode=DR)
                for gi in range(2):
                    base = (m % 2) * 1024 + 510 * gi + 1 + ph
                    nc.vector.scalar_tensor_tensor(
                        out=apn(d1p8[m // 2], base, (102, 5), (2, 50)),
                        in0=apn(psd[gi], 0, (52, 5), (1, 50)),
                        scalar=1.0 / SD1,
                        in1=apn(ertm, gi * 5, (1, 5), (0, 50)),
                        op0=ALU.mult, op1=ALU.add)
                    bcol = 0 if ph == 0 else Lq - 1
                    nc.vector.scalar_tensor_tensor(
                        out=apn(d1p8[m // 2], base + 2 * bcol, (102, 5), (2, 1)),
                        in0=apn(psd[gi], bcol, (52, 5), (1, 1)),
                        scalar=1.0 / SD1,
                        in1=apn(er2, gi * 5, (1, 5), (0, 1)),
                        op0=ALU.mult, op1=ALU.add)

        # ============ deconv2: d1[512,100] -> d2[256,200] ===================
        for m in range(MD2):
            for ph in range(2):
                psd = [pp.tile([128, 508], f32, tag="big", name="big", bufs=4)
                       for _ in range(2)]
                for ti, (t, off) in enumerate(TAPS[ph]):
                    for kp in range(2):
                        lhs = apn(wd2sb[kp], t * 256 + m * 128, (1024, 2), (1, 128))
                        for gi in range(2):
                            nc.tensor.matmul(
                                psd[gi], lhsT=lhs,
                                rhs=apn(d1p8[kp], 510 * gi + off, (1024, 2), (1, 508)),
                                start=(ti == 0 and kp == 0), stop=(ti == 1 and kp == 1),
                                perf_mode=DR)
                for gi in range(2):
                    nc.scalar.activation(
                        out=apn(d2sb[m], (5 * gi) * 200 + ph, (200, 5), (2, 100)),
                        in_=apn(psd[gi], 0, (102, 5), (1, 100)),
                        func=AF.Identity, bias=bd2[m], scale=1.0 / SD2)

        # ============ folded final projection + sigmoid =====================
        for p in range(PAIRS):
            fp = pp.tile([2, 400], f32, tag="tiny", name="tiny", bufs=1)
            for k in range(2):
                nc.tensor.matmul(fp, lhsT=vm[k],
                                 rhs=d2sb[k][:, 400 * p:400 * (p + 1)],
                                 start=(k == 0), stop=(k == 1))
            fo = tp.tile([2, 400], f32, tag="fout", name="fout", bufs=5)
            nc.scalar.activation(out=fo, in_=fp, func=AF.Sigmoid,
                                 bias=bmlp, scale=1.0)
            nc.sync.dma_start(out=d_out[:, 400 * p:400 * (p + 1)], in_=fo)

    nc.compile()   # bacc legalization: splits sync waits to <=1 per inst
    return nc


def _prep_inputs(batch, seg_len, concept1, concept2,
                 w_conv1, b_conv1, w_conv2, b_conv2,
                 w_ca1, w_ca2, w_ca3,
                 w_dc1, b_dc1, w_dc2, b_dc2,
                 w_sim1, w_sim2, w_mlp, b_mlp):
    f32 = np.float32

    # x: [B,M,L,IN_C] -> per core [PAIRS, 128, (kp8, k01, s01, 208)] fp8
    bm = np.ascontiguousarray(batch, f32).reshape(B * M, L, IN_C)
    bt = bm.transpose(0, 2, 1)                          # [80, 2048, 200]
    X = np.zeros((B * M, 16, 128, 208), F8)
    X[:, :, :, 2:202] = bt.reshape(B * M, 16, 128, L).astype(F8)
    xw = X.reshape(NCORES, PAIRS, 2, 8, 2, 128, 208) \
          .transpose(0, 1, 5, 3, 4, 2, 6).reshape(NCORES, PAIRS, 128, 6656)
    xw = np.ascontiguousarray(xw)

    # DoubleRow weight layouts: [.., ci(128), (k01, taps, m, co)], scaled
    w1p = np.ascontiguousarray(
        (np.asarray(w_conv1, f32) * S1).reshape(M1, 128, 8, 2, 128, 5)
        .transpose(2, 4, 3, 5, 0, 1).reshape(8, 128, 5120)).astype(F8)
    w2p = np.ascontiguousarray(
        (np.asarray(w_conv2, f32) * S2).reshape(M2, 128, 2, 2, 128, 5)
        .transpose(2, 4, 3, 5, 0, 1).reshape(2, 128, 2560)).astype(F8)
    wd1_ = np.asarray(w_dc1, f32)
    wd1p = np.ascontiguousarray(
        (wd1_[:256] * SD1).reshape(2, 128, MD1, 128, 4)
        .transpose(1, 0, 4, 2, 3).reshape(128, 4096)).astype(F8)
    wd2p = np.ascontiguousarray(
        (np.asarray(w_dc2, f32) * SD2).reshape(2, 2, 128, MD2, 128, 4)
        .transpose(0, 2, 1, 5, 3, 4).reshape(2, 128, 2048)).astype(F8)
    # summed-tap / correction-tap deconv1 weights for the rank-1 r-channels
    wr = wd1_[256:768].reshape(4, 128, MD1, 128, 4)     # [rk, ci, m, co, t]
    wsum = np.ascontiguousarray(
        (np.stack([wr[..., 1] + wr[..., 3], wr[..., 2] + wr[..., 0]], 0) * SD1)
        .transpose(2, 0, 1, 3, 4).reshape(128, 4096)).astype(F8)
    wcorr = np.ascontiguousarray(
        (np.stack([wr[..., 3], wr[..., 0]], 0) * SD1)
        .transpose(2, 0, 1, 3, 4).reshape(128, 4096)).astype(F8)
    wca2p = np.ascontiguousarray(
        (np.asarray(w_ca2, f32).T * SA).reshape(2, 128, 256)).astype(F8)
    wca3t = np.asarray(w_ca3, f32)[0].reshape(2, 128, 1).astype(BF16)
    b1s = (S1 * np.asarray(b_conv1, f32)).reshape(M1, 128, 1)
    b2v = np.asarray(b_conv2, f32).reshape(M2, 128, 1)
    bd1v = np.asarray(b_dc1, f32).reshape(MD1, 128, 1)
    bd2v = np.asarray(b_dc2, f32).reshape(MD2, 128, 1)
    bmlp = np.full((2, 1), np.asarray(b_mlp, f32).reshape(-1)[0], f32)

    # per-core mask / q / v
    nvalid = ((np.asarray(seg_len) + 3) // 4).reshape(B * M)
    amask = np.where(np.arange(Lq)[None, :] < nvalid[:, None], 0.0, NEG) \
        .astype(f32).reshape(NCORES, PAIRS, 1, 2 * Lq)
    concepts = [np.asarray(concept1, f32), np.asarray(concept2, f32)]
    w_ca1 = np.asarray(w_ca1, f32)
    w_sim1 = np.asarray(w_sim1, f32)
    w_sim2 = np.asarray(w_sim2, f32)
    wm = np.asarray(w_mlp, f32)[0]
    qv_all = np.zeros((NCORES, 4, 128, 1), f32)
    v_all = np.zeros((NCORES, 2, 128, 2), f32)
    for core in range(NCORES):
        bidx = (core * SEQ) // M
        for c in range(2):
            q = w_ca1 @ concepts[c][bidx]                       # [256]
            qv_all[core, 2 * c:2 * c + 2] = q.reshape(2, 128, 1)
            v = w_sim1.T @ ((w_sim2 @ concepts[c][bidx]) * wm)  # [256]
            v_all[core, :, :, c] = v.reshape(2, 128)
    vmat = v_all.astype(BF16)

    shared = dict(w1=w1p, w2=w2p, wd1=wd1p, wd2=wd2p, wsum=wsum, wcorr=wcorr,
                  wca2=wca2p, wca3=wca3t, b1s=b1s, b2=b2v, bd1=bd1v, bd2=bd2v,
                  bmlp=bmlp)
    return [dict(shared, xw=xw[c], amask=amask[c], qv=qv_all[c], vmat=vmat[c])
            for c in range(NCORES)]


_CACHE = {}


def kernel(**inputs):
    from concourse.bass_utils import run_bass_kernel_spmd

    in_maps = _prep_inputs(**inputs)
    if "nc" not in _CACHE:
        _CACHE["nc"] = _build_program()
    res = run_bass_kernel_spmd(_CACHE["nc"], in_maps, list(range(NCORES)))
    out = np.stack([np.asarray(r["out"], np.float32) for r in res.results])
    sc = out.transpose(1, 0, 2).reshape(2, B, M, L)
    return sc[0], sc[1]
